# revision 30
# baseline (speedup 1.0000x reference)
"""Trainium2 Bass kernel for the attention+global-LN+MoE(top2)+global-LN block.

Three launches on 8 cores (each core owns 512 rows = one quarter of one sample):
  A: attention in fp8 DoubleRow (QKV projections) + bf16 (scores, W2),
     column-parallel over heads.  Emits the raw attention output a^T (bf16,
     no residual/bias).  The scrambled [h,dh,N]->[N,h*dh] view is realized
     with a bf16 DRAM round-trip + XBAR transpose DMAs.
  host: y1 = a + b2 + x; global LN1 stats (f64); router top-2; x1 = LN1(y1);
     fp8 quantization and expert weight packing.
  B: MoE over the 2 selected experts, all-fp8 DoubleRow.  fc and proj are
     interleaved per k-pair (proj runs one pair behind fc's gelu) so the PE
     never drains; PSUM holds 6 proj accumulators + 2 fc banks.  Emits only
     the raw MoE matmul output m^T (bf16).  No stats, no collective.
  host: y2 = m + bc + x1; global LN2 stats (f64); x1s = (x1+bc)*s2+sh2.
  C: out = m*s2 + x1s (tiny elementwise launch).
"""

import numpy as np
import ml_dtypes

import concourse.bass as bass
from concourse import bacc
import concourse.mybir as mybir
import concourse.tile as tile
from concourse.bass_utils import run_bass_kernel_spmd
from concourse.masks import make_identity

F32 = mybir.dt.float32
F8 = mybir.dt.float8e4
BF16 = mybir.dt.bfloat16
AF = mybir.ActivationFunctionType
AX = mybir.AxisListType
ALU = mybir.AluOpType
DR = mybir.MatmulPerfMode.DoubleRow

NP_F8 = ml_dtypes.float8_e4m3
NP_BF16 = ml_dtypes.bfloat16

B, N, D, E = 2, 2048, 768, 8
H = 4 * D
NH = 12
DH = 64
TOP_K = 2
P = 128
ROWS = 512
HPC = 3
EPS = 1e-12
M_TOT = B * N * D
INV_SQRT_N = 1.0 / float(np.sqrt(np.float32(N)))
DC = D // P          # 6 channel chunks
KH = H // P          # 24 hidden chunks per expert
WS = 64.0            # fp8 attention/fc weight prescale
PS = 256.0           # fp8 proj weight prescale (gate val folded in)

N_CORES = 8


def _r(ap, pat, **kw):
    return ap.rearrange(pat, **kw)


# ---------------------------------------------------------------- launch A ---
def build_launch_a():
    nc = bacc.Bacc(None, target_bir_lowering=False, debug=False)
    xT = nc.declare_dram_parameter("xT", [4, P, DC, 512], F8, isOutput=False)
    w1qk = nc.declare_dram_parameter("w1qk", [P, DC, 384], F8, isOutput=False)
    b1qk = nc.declare_dram_parameter("b1qk", [P, 384], F32, isOutput=False)
    w1v = nc.declare_dram_parameter("w1v", [P, DC, 192], F8, isOutput=False)
    b1v = nc.declare_dram_parameter("b1v", [P, 2], F32, isOutput=False)
    w2 = nc.declare_dram_parameter("w2", [P, DC, D], BF16, isOutput=False)
    aT_out = nc.declare_dram_parameter("aT", [P, DC, ROWS], BF16, isOutput=True)

    o_dram = nc.dram_tensor("o_scratch", [ROWS, D], BF16)
    o_flat = _r(_r(o_dram[:], "a c -> (a c)"), "(h d n) -> d h n", h=HPC, d=DH)

    with tile.TileContext(nc) as tc:
        with (
            tc.tile_pool(name="const", bufs=1) as const,
            tc.tile_pool(name="persist", bufs=1) as persist,
            tc.tile_pool(name="small", bufs=4) as small,
        ):
            ident = const.tile([64, 64], F32)
            make_identity(nc, ident)
            identb = const.tile([P, P], BF16)
            make_identity(nc, identb)
            # gating loads split across both queues; bulk follows
            w1qk_sb = const.tile([P, DC, 384], F8)
            nc.sync.dma_start(out=w1qk_sb[:], in_=w1qk[:])
            xT_c = [const.tile([P, DC, 512], F8, tag=f"xt{f}", name=f"xt_t{f}")
                    for f in range(4)]
            nc.scalar.dma_start(out=xT_c[0][:], in_=xT[0])
            b1qk_sb = const.tile([P, 384], F32)
            nc.scalar.dma_start(out=b1qk_sb[:], in_=b1qk[:])
            for f in range(1, 4):
                nc.sync.dma_start(out=xT_c[f][:], in_=xT[f])
            w1v_sb = const.tile([P, DC, 192], F8)
            nc.scalar.dma_start(out=w1v_sb[:], in_=w1v[:])
            b1v_sb = const.tile([P, 2], F32)
            nc.scalar.dma_start(out=b1v_sb[:], in_=b1v[:])
            w2_sb = persist.tile([P, DC, D], BF16)
            nc.sync.dma_start(out=w2_sb[:], in_=w2[:])

            # PE clock warm-up on junk data while the input DMAs land
            wu_l = const.tile([P, 2, 64], F8)
            wu_r = const.tile([P, 2, 64], F8)
            nc.vector.memset(wu_l[:], 0.0)
            nc.vector.memset(wu_r[:], 0.0)

            qk_sb = persist.tile([P, 16, 384], BF16)
            vt_sb = persist.tile([P, 2, N], BF16)
            ovt_sb = persist.tile([P, DC, ROWS], BF16)

            with tc.tile_pool(name="psA", bufs=2, space="PSUM") as psA:
                for i in range(13):
                    ps = psA.tile([P, 384], F32, tag="qk")
                    nc.tensor.matmul(ps[:64, :64], wu_l[:], wu_r[:],
                                     start=True, stop=True, perf_mode=DR)
                # phase 1: Q,K = x @ W1qk (fp8 DoubleRow), carries WS scale
                for m in range(16):
                    c, mi = divmod(m, 4)
                    ps = psA.tile([P, 384], F32, tag="qk")
                    for i, b in enumerate((0, 2, 4)):
                        nc.tensor.matmul(
                            ps[:],
                            xT_c[c][:, b:b + 2, mi * P:(mi + 1) * P],
                            w1qk_sb[:, b:b + 2, :],
                            start=(i == 0),
                            stop=(i == 2),
                            perf_mode=DR,
                        )
                    nc.vector.tensor_add(qk_sb[:, m, :], ps[:], b1qk_sb[:])

                # phase 2: V^T = W1v^T @ x^T (fp8 DoubleRow), carries WS
                for mo in range(2):
                    mp = P if mo == 0 else 64
                    for f in range(4):
                        ps = psA.tile([P, 512], F32, tag="vt")
                        for i, b in enumerate((0, 2, 4)):
                            nc.tensor.matmul(
                                ps[:mp],
                                w1v_sb[:, b:b + 2, mo * P: mo * P + mp],
                                xT_c[f][:, b:b + 2, :],
                                start=(i == 0),
                                stop=(i == 2),
                                perf_mode=DR,
                            )
                        if f % 2 == 0:
                            nc.vector.tensor_scalar(
                                out=vt_sb[:mp, mo, f * 512:(f + 1) * 512],
                                in0=ps[:mp], scalar1=b1v_sb[:mp, mo: mo + 1],
                                scalar2=None, op0=ALU.add,
                            )
                        else:
                            nc.scalar.activation(
                                out=vt_sb[:mp, mo, f * 512:(f + 1) * 512],
                                in_=ps[:mp], func=AF.Identity,
                                bias=b1v_sb[:mp, mo: mo + 1],
                            )

            # phase 3: per-head scores/softmax/O; score logits carry WS^2
            sc_scale = INV_SQRT_N / (WS * WS)
            with (
                tc.tile_pool(name="op", bufs=1) as op,
                tc.tile_pool(name="ovp", bufs=4) as ovp,
                tc.tile_pool(name="psB", bufs=2, space="PSUM") as psB,
            ):
                o_sb = op.tile([64, HPC, N], BF16)
                # all scores first, then the 3 softmax chains interleaved
                # stage-by-stage so the Act/DVE ping-pong pipelines
                ps_sc = [psB.tile([64, 64], F32, tag=f"sc{h}", bufs=1,
                                  name=f"ps_sc{h}") for h in range(HPC)]
                for h in range(HPC):
                    for m in range(16):
                        nc.tensor.matmul(
                            ps_sc[h][:],
                            qk_sb[:, m, h * 64:(h + 1) * 64],
                            qk_sb[:, m, 192 + h * 64: 192 + (h + 1) * 64],
                            start=(m == 0),
                            stop=(m == 15),
                        )
                mx = [small.tile([64, 1], F32, tag=f"mx{h}", name=f"mx{h}")
                      for h in range(HPC)]
                negmx = [small.tile([64, 1], F32, tag=f"ng{h}", name=f"ng{h}")
                         for h in range(HPC)]
                w_sm = [small.tile([64, 64], F32, tag=f"ws{h}", name=f"ws{h}")
                        for h in range(HPC)]
                sm = [small.tile([64, 1], F32, tag=f"sm{h}", name=f"sm{h}")
                      for h in range(HPC)]
                rinv = [small.tile([64, 1], F32, tag=f"ri{h}", name=f"ri{h}")
                        for h in range(HPC)]
                wt_sb = [small.tile([P, 64], BF16, tag=f"wt{h}", name=f"wt{h}")
                         for h in range(HPC)]
                for h in range(HPC):
                    nc.vector.reduce_max(out=mx[h][:], in_=ps_sc[h][:], axis=AX.X)
                for h in range(HPC):
                    nc.scalar.activation(out=negmx[h][:], in_=mx[h][:],
                                         func=AF.Copy, scale=-sc_scale)
                for h in range(HPC):
                    nc.scalar.activation(out=w_sm[h][:], in_=ps_sc[h][:],
                                         func=AF.Exp, scale=sc_scale,
                                         bias=negmx[h][:])
                for h in range(HPC):
                    nc.vector.reduce_sum(out=sm[h][:], in_=w_sm[h][:], axis=AX.X)
                for h in range(HPC):
                    nc.vector.reciprocal(out=rinv[h][:], in_=sm[h][:])
                for h in range(HPC):
                    nc.vector.tensor_scalar_mul(w_sm[h][:], w_sm[h][:], rinv[h][:])
                for h in range(HPC):
                    # transpose back into the (already consumed) score bank
                    nc.tensor.transpose(ps_sc[h][:], w_sm[h][:], ident[:])
                for h in range(HPC):
                    off = 64 if h == 1 else 0
                    if off == 0:
                        nc.scalar.copy(wt_sb[h][0:64, :], ps_sc[h][:])
                    else:
                        wt_tmp = small.tile([64, 64], BF16, tag="wttmp")
                        nc.vector.tensor_copy(wt_tmp[:], ps_sc[h][:])
                        nc.sync.dma_start(out=wt_sb[h][64:128, :], in_=wt_tmp[:])
                for h in range(HPC):
                    off = 64 if h == 1 else 0
                    vchunk = 0 if h < 2 else 1
                    for f in range(4):
                        ps_o = psB.tile([64, 512], F32, tag="o")
                        nc.tensor.matmul(
                            ps_o[:],
                            wt_sb[h][off:off + 64, :],
                            vt_sb[off:off + 64, vchunk, f * 512:(f + 1) * 512],
                            start=True,
                            stop=True,
                        )
                        if f % 2 == 0:
                            nc.vector.tensor_copy(
                                o_sb[:, h, f * 512:(f + 1) * 512], ps_o[:])
                        else:
                            nc.scalar.copy(
                                o_sb[:, h, f * 512:(f + 1) * 512], ps_o[:])
                    nc.sync.dma_start(out=o_flat[:, h, :], in_=o_sb[:, h, :])

                    # phase 5 (rolling): re-read the view rows this head
                    # completes and PE-transpose them into ovT
                    ready = {0: (0,), 1: (1,), 2: (2, 3)}[h]
                    for a in ready:
                        ov_t = ovp.tile([P, D], BF16, tag="ov")
                        nc.sync.dma_start(
                            out=ov_t[:], in_=o_dram[a * P:(a + 1) * P, :])
                        for bb in range(DC):
                            ps_t = psB.tile([P, P], BF16, tag="ts", bufs=3)
                            nc.tensor.transpose(
                                ps_t[:], ov_t[:, bb * P:(bb + 1) * P],
                                identb[:])
                            if bb % 2 == 0:
                                nc.vector.tensor_copy(
                                    ovt_sb[:, bb, a * P:(a + 1) * P], ps_t[:])
                            else:
                                nc.scalar.copy(
                                    ovt_sb[:, bb, a * P:(a + 1) * P], ps_t[:])

            # phase 6: aT = (W2^T @ ovT)/WS  (b2 folded in on host).
            # b-outer accumulation over 6 PSUM banks so the first matmul
            # only needs ovt strip b=0, not all six; the last b-round is
            # dc-ordered with immediate per-dc finalize.
            with (
                tc.tile_pool(name="yp", bufs=3) as yp,
                tc.tile_pool(name="psD", bufs=1, space="PSUM") as psD,
            ):
                ps_y = [psD.tile([P, 512], F32, tag=f"y{dc}",
                                 name=f"ps_y{dc}") for dc in range(DC)]
                for b in range(DC - 1):
                    for dc in range(DC):
                        nc.tensor.matmul(
                            ps_y[dc][:],
                            w2_sb[:, b, dc * P:(dc + 1) * P],
                            ovt_sb[:, b, :],
                            start=(b == 0),
                            stop=False,
                        )
                for dc in range(DC):
                    nc.tensor.matmul(
                        ps_y[dc][:],
                        w2_sb[:, 5, dc * P:(dc + 1) * P],
                        ovt_sb[:, 5, :],
                        start=False,
                        stop=True,
                    )
                    y_sb = yp.tile([P, 512], BF16, tag="ytile")
                    if dc % 2 == 0:
                        nc.vector.tensor_scalar(
                            out=y_sb[:], in0=ps_y[dc][:], scalar1=1.0 / WS,
                            scalar2=None, op0=ALU.mult,
                        )
                    else:
                        nc.scalar.activation(out=y_sb[:], in_=ps_y[dc][:],
                                             func=AF.Copy, scale=1.0 / WS)
                    eng = nc.sync if dc % 2 == 0 else nc.scalar
                    eng.dma_start(out=aT_out[:, dc, :], in_=y_sb[:])
    nc.compile()
    return nc


# ---------------------------------------------------------------- launch B ---
def build_launch_b():
    nc = bacc.Bacc(None, target_bir_lowering=False, debug=False)
    x1q = nc.declare_dram_parameter("x1q", [P, DC, ROWS], F8, isOutput=False)
    # fc weights: per expert, per 512-col group: [128, ksub(6), 512], * WS
    fcw = nc.declare_dram_parameter("fcw", [2, 6, P, DC, 512], F8, isOutput=False)
    fcb = nc.declare_dram_parameter("fcb", [2, P, KH], F32, isOutput=False)
    # proj weights per k-pair (pp = e*12 + p): [128, 2, 768], * PS * gate_val
    pjw = nc.declare_dram_parameter("pjw", [KH, P, 2, D], F8, isOutput=False)
    mT_out = nc.declare_dram_parameter("mT", [P, DC, ROWS], BF16, isOutput=True)

    with tile.TileContext(nc) as tc:
        with (
            tc.tile_pool(name="const", bufs=1) as const,
            tc.tile_pool(name="wts", bufs=1) as wts,
            tc.tile_pool(name="hmp", bufs=3) as hmp,
            tc.tile_pool(name="fin", bufs=2) as fin,
            tc.tile_pool(name="psfc", bufs=2, space="PSUM") as psfc,
            tc.tile_pool(name="psacc", bufs=1, space="PSUM") as psacc,
        ):
            # ALL bulk weights ride the sync (SP) queue in consumption order:
            # DMA triggers on the Activation engine's queue would block the
            # gelu stream behind a full DGE ring.  The scalar queue carries
            # only the tiny early loads.
            x1q_sb = const.tile([P, DC, ROWS], F8)
            nc.sync.dma_start(out=x1q_sb[:, 0:3, :], in_=x1q[:, 0:3, :])
            nc.scalar.dma_start(out=x1q_sb[:, 3:6, :], in_=x1q[:, 3:6, :])
            fcb_sb = [const.tile([P, KH], F32, tag=f"fcb{e}",
                                 name=f"fcb_sb{e}") for e in range(2)]
            nc.scalar.dma_start(out=fcb_sb[0][:], in_=fcb[0])
            nc.scalar.dma_start(out=fcb_sb[1][:], in_=fcb[1])
            fcw_sb = [[wts.tile([P, DC, 512], F8, tag=f"fcw{e}_{g}",
                                name=f"fcw_sb{e}_{g}") for g in range(6)]
                      for e in range(2)]
            pjw_sb = [wts.tile([P, 2, D], F8, tag=f"pjw{pp}",
                               name=f"pjw_sb{pp}") for pp in range(KH)]
            nc.sync.dma_start(out=fcw_sb[0][0][:], in_=fcw[0, 0])
            # interleave fcw group g (feeds pairs 2g,2g+1) with the pjw
            # pairs consumed alongside it
            for e in range(2):
                for g in range(6):
                    if (e, g) != (0, 0):
                        nc.sync.dma_start(out=fcw_sb[e][g][:], in_=fcw[e, g])
                    base = e * 12 + 2 * g
                    for pp in (base - 1, base):
                        if 0 <= pp < KH:
                            nc.sync.dma_start(out=pjw_sb[pp][:], in_=pjw[pp])
            nc.sync.dma_start(out=pjw_sb[23][:], in_=pjw[23])

            ps_o = [psacc.tile([P, ROWS], F32, tag=f"acc{dc}", name=f"ps_o{dc}")
                    for dc in range(DC)]

            # warm-up matmuls on junk data while input DMAs land: keeps the
            # PE busy from t~1us so the clock is ramped before real work
            wu_l = const.tile([P, 2, 64], F8)
            wu_r = const.tile([P, 2, 64], F8)
            nc.vector.memset(wu_l[:], 0.0)
            nc.vector.memset(wu_r[:], 0.0)
            for i in range(12):
                ps = psfc.tile([P, ROWS], F32, tag="fc")
                nc.tensor.matmul(ps[:64, :64], wu_l[:], wu_r[:],
                                 start=True, stop=True, perf_mode=DR)

            def fc_pair(e, p):
                hm_t = hmp.tile([P, 2, ROWS], F8, tag="hm")
                for half in range(2):
                    fo = 2 * p + half
                    g, fg = divmod(fo, 4)
                    ps = psfc.tile([P, ROWS], F32, tag="fc")
                    for i, b in enumerate((0, 2, 4)):
                        nc.tensor.matmul(
                            ps[:],
                            fcw_sb[e][g][:, b:b + 2, fg * P:(fg + 1) * P],
                            x1q_sb[:, b:b + 2, :],
                            start=(i == 0),
                            stop=(i == 2),
                            perf_mode=DR,
                        )
                    nc.scalar.activation(out=hm_t[:, half, :], in_=ps[:],
                                         func=AF.Gelu_apprx_tanh,
                                         scale=1.0 / WS,
                                         bias=fcb_sb[e][:, fo: fo + 1])
                return hm_t

            def proj_pair(pp, hm_t, start, stop):
                for dc in range(DC):
                    nc.tensor.matmul(
                        ps_o[dc][:],
                        pjw_sb[pp][:, :, dc * P:(dc + 1) * P],
                        hm_t[:, :, :],
                        start=start,
                        stop=stop,
                        perf_mode=DR,
                    )

            # unified loop: fc(pair pp) then proj(pair pp-1); the PE stays
            # saturated while the gelu of pair pp runs on the Act engine
            hm_prev = None
            for pp in range(2 * 12):
                e, p = divmod(pp, 12)
                hm_t = fc_pair(e, p)
                if hm_prev is not None:
                    proj_pair(pp - 1, hm_prev, start=(pp == 1), stop=False)
                hm_prev = hm_t
            proj_pair(23, hm_prev, start=False, stop=True)

            # finalize: m = ps/PS -> bf16 (bias+residual+LN2 applied later)
            m_sb = const.tile([P, DC, ROWS], BF16)
            for dc in range(DC):
                if dc % 2 == 0:
                    nc.vector.tensor_scalar(
                        out=m_sb[:, dc, :], in0=ps_o[dc][:],
                        scalar1=1.0 / PS, scalar2=None, op0=ALU.mult,
                    )
                else:
                    nc.scalar.activation(out=m_sb[:, dc, :], in_=ps_o[dc][:],
                                         func=AF.Copy, scale=1.0 / PS)
                eng = nc.sync if dc % 2 == 0 else nc.scalar
                eng.dma_start(out=mT_out[:, dc, :], in_=m_sb[:, dc, :])
    nc.compile()
    return nc


# ---------------------------------------------------------------- launch C ---
def build_launch_c():
    nc = bacc.Bacc(None, target_bir_lowering=False, debug=False)
    mT = nc.declare_dram_parameter("mT", [P, DC, ROWS], BF16, isOutput=False)
    x1s = nc.declare_dram_parameter("x1s", [P, DC, ROWS], BF16, isOutput=False)
    s2 = nc.declare_dram_parameter("s2", [P, DC], F32, isOutput=False)
    outT = nc.declare_dram_parameter("outT", [P, DC, ROWS], F32, isOutput=True)

    with tile.TileContext(nc) as tc:
        with (
            tc.tile_pool(name="sb", bufs=1) as sb,
            tc.tile_pool(name="outp", bufs=6) as outp,
        ):
            s2_sb = sb.tile([P, DC], F32)
            nc.scalar.dma_start(out=s2_sb[:], in_=s2[:])
            m_sb = sb.tile([P, DC, ROWS], BF16)
            x1s_sb = sb.tile([P, DC, ROWS], BF16)
            # 2-way chunked loads: few triggers (HWDGE triggers cost the
            # issuing engine ~1us each) but the first half arrives early so
            # the Copy chain can start sooner
            nc.sync.dma_start(out=m_sb[:, 0:3, :], in_=mT[:, 0:3, :])
            nc.scalar.dma_start(out=m_sb[:, 3:6, :], in_=mT[:, 3:6, :])
            nc.sync.dma_start(out=x1s_sb[:, 0:3, :], in_=x1s[:, 0:3, :])
            nc.scalar.dma_start(out=x1s_sb[:, 3:6, :], in_=x1s[:, 3:6, :])
            for dc in range(DC):
                tmp = outp.tile([P, ROWS], F32, tag="tmp")
                nc.scalar.activation(out=tmp[:], in_=m_sb[:, dc, :],
                                     func=AF.Copy,
                                     scale=s2_sb[:, dc: dc + 1])
                ot = outp.tile([P, ROWS], F32, tag="ot")
                nc.vector.tensor_add(ot[:], tmp[:], x1s_sb[:, dc, :])
                nc.sync.dma_start(out=outT[:, dc, :], in_=ot[:])
    nc.compile()
    return nc


# ------------------------------------------------------------------- host ---
_CACHE = {}
PROFILE = False
LAST_EXEC_NS = {}


def _get_nc(which):
    if which not in _CACHE:
        _CACHE[which] = {"a": build_launch_a, "b": build_launch_b,
                         "c": build_launch_c}[which]()
    return _CACHE[which]


def _softmax_np(x):
    x = x - x.max()
    e = np.exp(x)
    return e / e.sum()


def _run(which, in_maps):
    kwargs = {}
    if PROFILE:
        kwargs = dict(trace=True)
    res = run_bass_kernel_spmd(_get_nc(which), in_maps, list(range(N_CORES)),
                               **kwargs)
    if res.exec_time_ns is not None:
        LAST_EXEC_NS[which] = res.exec_time_ns
    return res


def pack_po(a):
    """[K, F] -> [128, K//128, F] SBUF-layout pack (contiguous DMA)."""
    K_, F_ = a.shape
    return np.ascontiguousarray(
        a.reshape(K_ // P, P, F_).transpose(1, 0, 2))


def _unpack(aT):
    """[128, DC, ROWS] -> [ROWS, D] float32."""
    return aT.astype(np.float32).transpose(2, 1, 0).reshape(ROWS, D)


def pack_a_inputs(x, W1_w, W1_b, W2_w):
    f32 = lambda a: np.ascontiguousarray(a, np.float32)
    xT_pk = []
    for s in range(B):
        xTs = x[s].T.astype(NP_F8)
        xT_pk.append(np.stack([pack_po(xTs[:, c * 512:(c + 1) * 512])
                               for c in range(4)]))
    w2_pk = np.ascontiguousarray(pack_po(W2_w).astype(NP_BF16))
    in_maps_a = []
    for g in range(N_CORES):
        s, q = divmod(g, 4)
        h0 = HPC * q * DH
        w1qk = np.concatenate(
            [W1_w[:, h0:h0 + 192], W1_w[:, D + h0:D + h0 + 192]], 1)
        b1qk = np.broadcast_to(
            np.concatenate([W1_b[h0:h0 + 192], W1_b[D + h0:D + h0 + 192]])
            * np.float32(WS), (P, 384))
        bv = W1_b[2 * D + h0: 2 * D + h0 + 192] * np.float32(WS)
        b1v = np.zeros((P, 2), np.float32)
        b1v[:, 0] = bv[:P]
        b1v[:64, 1] = bv[P:]
        in_maps_a.append({
            "xT": xT_pk[s],
            "w1qk": np.ascontiguousarray(pack_po(w1qk * WS).astype(NP_F8)),
            "b1qk": f32(b1qk),
            "w1v": np.ascontiguousarray(
                pack_po(W1_w[:, 2 * D + h0: 2 * D + h0 + 192] * WS).astype(NP_F8)),
            "b1v": b1v,
            "w2": w2_pk,
        })
    return in_maps_a


def pack_b_inputs(x1, sel, fc_w, fc_b, proj_w):
    """x1: [B, N, D] float32 (LN1 applied on host)."""
    sample_packs = []
    fcw8_cache = {}
    for s in range(B):
        idx, gv = sel[s]
        fcw_parts, fcb_parts, pjw_parts = [], [], []
        for e in range(TOP_K):
            ex = int(idx[e])
            if ex not in fcw8_cache:
                w8 = (fc_w[ex] * WS).astype(NP_F8)
                fcw8_cache[ex] = np.stack(
                    [pack_po(w8[:, g * 512:(g + 1) * 512]) for g in range(6)])
            fcw_parts.append(fcw8_cache[ex])
            fcb_parts.append(
                np.ascontiguousarray(fc_b[ex].reshape(KH, P).T, np.float32))
            pw = (proj_w[ex] * (PS * float(gv[e]))).astype(NP_F8)
            pjw_parts.append(
                pw.reshape(12, 2, P, D).transpose(0, 2, 1, 3))
        sample_packs.append({
            "fcw": np.stack(fcw_parts),                    # [2,6,128,6,512]
            "fcb": np.stack(fcb_parts),                    # [2,128,24]
            "pjw": np.ascontiguousarray(
                np.concatenate(pjw_parts, axis=0)),        # [24,128,2,768]
        })
    in_maps_b = []
    for g in range(N_CORES):
        s, q = divmod(g, 4)
        x1T = x1[s, q * ROWS:(q + 1) * ROWS, :].T       # [768, 512]
        x1p = x1T.reshape(DC, P, ROWS).transpose(1, 0, 2)
        im = dict(sample_packs[s])
        im["x1q"] = np.ascontiguousarray(x1p.astype(NP_F8))
        in_maps_b.append(im)
    return in_maps_b


def kernel(x, W1_w, W1_b, W2_w, W2_b, r_w, r_b, fc_w, fc_b, proj_w, proj_b,
           ln1_w, ln1_b, ln2_w, ln2_b):
    x = np.asarray(x, np.float32)
    W1_w = np.asarray(W1_w, np.float32)
    W1_b = np.asarray(W1_b, np.float32)
    W2_w = np.asarray(W2_w, np.float32)
    W2_b = np.asarray(W2_b, np.float32)
    r_w = np.asarray(r_w, np.float32)
    r_b = np.asarray(r_b, np.float32)
    fc_w = np.asarray(fc_w, np.float32)
    fc_b = np.asarray(fc_b, np.float32)
    proj_w = np.asarray(proj_w, np.float32)
    proj_b = np.asarray(proj_b, np.float32)
    ln1_w = np.asarray(ln1_w, np.float32)
    ln1_b = np.asarray(ln1_b, np.float32)
    ln2_w = np.asarray(ln2_w, np.float32)
    ln2_b = np.asarray(ln2_b, np.float32)

    res_a = _run("a", pack_a_inputs(x, W1_w, W1_b, W2_w))

    # y1 = a + b2 + x ; global LN1 (f64) ; router top-2
    y1 = np.empty((B, N, D), np.float32)
    for g in range(N_CORES):
        s, q = divmod(g, 4)
        y1[s, q * ROWS:(q + 1) * ROWS, :] = _unpack(res_a.results[g]["aT"])
    y1 += W2_b[None, None, :]
    y1 += x
    m1 = y1.mean(dtype=np.float64)
    v1 = y1.var(ddof=1, dtype=np.float64)
    rstd1 = 1.0 / np.sqrt(v1 + EPS)
    scale_c = ln1_w.astype(np.float64) * rstd1
    shift_c = ln1_b.astype(np.float64) - m1 * scale_c

    sel = []
    for s in range(B):
        mean_x1 = y1[s].mean(axis=0, dtype=np.float64) * scale_c + shift_c
        logits = mean_x1 @ r_w.astype(np.float64) + r_b.astype(np.float64)
        gate = _softmax_np(logits)
        idx = np.argsort(-gate, kind="stable")[:TOP_K]
        sel.append((idx, gate[idx]))

    x1 = (y1 * scale_c.astype(np.float32)[None, None, :]
          + shift_c.astype(np.float32)[None, None, :])

    res_b = _run("b", pack_b_inputs(x1, sel, fc_w, fc_b, proj_w))

    # y2 = m + bc + x1 ; global LN2 (f64) on host
    bc = np.empty((B, D), np.float64)
    for s in range(B):
        idx, gv = sel[s]
        bc[s] = sum(float(gv[e]) * proj_b[int(idx[e])].astype(np.float64)
                    for e in range(TOP_K))
    m_rows = [_unpack(res_b.results[g]["mT"]) for g in range(N_CORES)]
    S2 = 0.0
    SQ2 = 0.0
    for g in range(N_CORES):
        s, q = divmod(g, 4)
        y2g = (m_rows[g] + bc[s].astype(np.float32)[None, :]
               + x1[s, q * ROWS:(q + 1) * ROWS, :]).astype(np.float64)
        S2 += y2g.sum()
        SQ2 += (y2g * y2g).sum()
    m2 = S2 / M_TOT
    v2 = (SQ2 - S2 * S2 / M_TOT) / (M_TOT - 1)
    rstd2 = 1.0 / np.sqrt(v2 + EPS)
    scale2 = ln2_w.astype(np.float64) * rstd2
    shift2 = ln2_b.astype(np.float64) - m2 * scale2
    s2_pack = np.ascontiguousarray(
        scale2.astype(np.float32).reshape(DC, P).T)

    in_maps_c = []
    for g in range(N_CORES):
        s, q = divmod(g, 4)
        x1s = ((x1[s, q * ROWS:(q + 1) * ROWS, :] + bc[s].astype(np.float32))
               * scale2.astype(np.float32) + shift2.astype(np.float32))
        x1sp = x1s.T.reshape(DC, P, ROWS).transpose(1, 0, 2)
        in_maps_c.append({
            "mT": res_b.results[g]["mT"],
            "x1s": np.ascontiguousarray(x1sp.astype(NP_BF16)),
            "s2": s2_pack,
        })
    res_c = _run("c", in_maps_c)

    out = np.empty((B, N, D), np.float32)
    for g in range(N_CORES):
        s, q = divmod(g, 4)
        oT = res_c.results[g]["outT"]
        out[s, q * ROWS:(q + 1) * ROWS, :] = (
            oT.transpose(2, 1, 0).reshape(ROWS, D))
    return out
